# revision 7
# baseline (speedup 1.0000x reference)
"""Trainium2 Bass kernel for DiagTrainableLDAHead (retrieval_knn).

out[n,c] = log_prior[c] - 0.5*(m2[n,c] + log_det)
m2[n,c]  = sum_d (z[n,d]-mu[c,d])^2 * inv_var[d]
         = z_sq[n] - 2*cross[n,c] + mu_sq[c]

=> out[n,c] = cross[n,c] + rb[n] + cb[c]
   cross = z @ w.T with w = mu * inv_var   (GEMM; fp8 DoubleRow, 2x rate)
   rb[n] = -0.5 * sum_d z[n,d]^2 inv_var[d]          (host, exact fp64)
   cb[c] = log_prior[c] - 0.5*(mu_sq[c] + log_det)   (host, exact fp64)

Sharding: data-parallel over N across 8 NeuronCores (1024 rows each);
w / biases replicated. Forward-only: no collectives.

Host prep: layout transposes, the O(N*D + C*D) bias vectors, and
quantization of the GEMM operands to e4m3 with power-of-two scales
(exact-in-binary scaling; the 1/(SZ*SW) back-scale is applied in the
PSUM-evict activation). The fp8 quantization error enters only through
cross products against the small mu values (|mu| ~ 0.19) because both
bias vectors are computed from the exact fp32 inputs, so max output
error stays ~1e-2 of the tolerance envelope.

Device schedule per core: inputs stream on both hwdge queues (z + half
of w on sync, other half of w + biases on scalar); each of 8 row blocks
runs 2 compound DoubleRow matmuls (one per k-pair, each lowering to a
single LDWEIGHTS + 4 bank-MATMULs) into a 4-bank PSUM tile; the evict
(ACT: scale + rb bias), the cb row add (DVE), and the store (sync
queue) are chunked at 1024 columns so the store queue starts draining
one chunk after the first matmul group finishes and never goes idle.
"""
import sys

sys.path.insert(0, "/opt/trn_rl_repo")

import numpy as np
import ml_dtypes

import concourse.bacc as bacc
import concourse.tile as tile
from concourse import mybir
from concourse.bass_utils import run_bass_kernel_spmd

F32 = mybir.dt.float32
F32R = mybir.dt.float32r
FP8 = mybir.dt.float8e4
AF = mybir.ActivationFunctionType
ALU = mybir.AluOpType
DR = mybir.MatmulPerfMode.DoubleRow

N, C, D = 8192, 2048, 512
NCORES = 8
NSH = N // NCORES          # 1024 rows per core
P = 128                    # partitions
KJ = D // P                # 4 k-tiles
NT = NSH // P              # 8 n-tiles
F = 512                    # PSUM bank width (fp32)
CJ = C // F                # 4 c-chunks
H = 1024                   # evict/store chunk width
CH = C // H                # 2 chunks per row block

_CACHE = {}


def _build():
    nc = bacc.Bacc("TRN2", target_bir_lowering=False, debug=False,
                   enable_asserts=False, num_devices=NCORES)

    z8 = nc.dram_tensor("z8", [D, NSH], FP8, kind="ExternalInput").ap()
    w8 = nc.dram_tensor("w8", [D, C], FP8, kind="ExternalInput").ap()
    rbt = nc.dram_tensor("rbt", [P, NT], F32, kind="ExternalInput").ap()
    cbr = nc.dram_tensor("cbr", [1, C], F32R, kind="ExternalInput").ap()
    sc = nc.dram_tensor("sc", [P, 1], F32, kind="ExternalInput").ap()
    out = nc.dram_tensor("out", [NSH, C], F32, kind="ExternalOutput").ap()

    with tile.TileContext(nc) as tc:
        with (
            tc.tile_pool(name="const", bufs=1) as const,
            tc.tile_pool(name="stage", bufs=3) as stage,
            tc.tile_pool(name="psM", bufs=2, space="PSUM") as psM,
        ):
            # ---- input loads: big streams split across both queues ----
            z8s = const.tile([P, KJ, NSH], FP8)
            w8s = const.tile([P, KJ, C], FP8)
            nc.sync.dma_start(out=z8s[:],
                              in_=z8.rearrange("(j p) n -> p j n", p=P))
            nc.scalar.dma_start(out=w8s[:, 0:2, :],
                                in_=w8[0:2 * P, :]
                                .rearrange("(j p) c -> p j c", p=P))
            nc.sync.dma_start(out=w8s[:, 2:4, :],
                              in_=w8[2 * P:4 * P, :]
                              .rearrange("(j p) c -> p j c", p=P))
            rbt_s = const.tile([P, NT], F32)
            nc.scalar.dma_start(out=rbt_s[:], in_=rbt[:, :])
            cbr_s = const.tile([1, C], F32R)
            nc.scalar.dma_start(out=cbr_s[:], in_=cbr[:, :])
            sc_s = const.tile([P, 1], F32)
            nc.scalar.dma_start(out=sc_s[:], in_=sc[:, :])

            # ---- cb broadcast [P, C] via rank-1 matmul ----------------
            ones_f = const.tile([1, P], F32)
            nc.vector.memset(ones_f[:], 1.0)
            ones1 = const.tile([1, P], F32R)
            nc.scalar.copy(ones1[:], ones_f[:])

            cb_b = const.tile([P, C], F32)
            ps0 = psM.tile([P, C], F32, tag="ps")
            for cj in range(CJ):
                nc.tensor.matmul(ps0[:, cj * F:(cj + 1) * F], lhsT=ones1[:],
                                 rhs=cbr_s[:, cj * F:(cj + 1) * F],
                                 start=True, stop=True)
            nc.vector.tensor_copy(cb_b[:], ps0[:])

            # ---- main loop: 8 row blocks, chunked output path ---------
            def main_tile(ni):
                ps = psM.tile([P, C], F32, tag="ps")
                for jj in range(2):
                    lhs = z8s[:, 2 * jj:2 * jj + 2, ni * P:(ni + 1) * P]
                    for cj in range(CJ):
                        nc.tensor.matmul(
                            ps[:, cj * F:(cj + 1) * F],
                            lhsT=lhs,
                            rhs=w8s[:, 2 * jj:2 * jj + 2, cj * F:(cj + 1) * F],
                            start=(jj == 0), stop=(jj == 1), perf_mode=DR)
                ot = stage.tile([P, C], F32)
                for h in range(CH):
                    s = slice(h * H, (h + 1) * H)
                    nc.scalar.activation(ot[:, s], ps[:, s], AF.Identity,
                                         bias=rbt_s[:, ni:ni + 1],
                                         scale=sc_s[:])
                    nc.vector.tensor_tensor(ot[:, s], ot[:, s], cb_b[:, s],
                                            ALU.add)
                    nc.sync.dma_start(out=out[ni * P:(ni + 1) * P, s],
                                      in_=ot[:, s])

            for ni in range(NT):
                main_tile(ni)

    nc.compile()
    return nc


def _get_nc():
    if "nc" not in _CACHE:
        _CACHE["nc"] = _build()
    return _CACHE["nc"]


def _pow2_scale(maxabs, limit=224.0):
    """Largest power of two s with maxabs * s <= limit (e4m3 max ~240)."""
    if maxabs <= 0 or not np.isfinite(maxabs):
        return 1.0
    return float(2.0 ** np.floor(np.log2(limit / maxabs)))


def _in_maps(z, mu, log_cov_diag, prior_logits):
    z = np.asarray(z, dtype=np.float32)
    mu = np.asarray(mu, dtype=np.float32)
    lc = np.asarray(log_cov_diag, dtype=np.float64)
    pl = np.asarray(prior_logits, dtype=np.float64)

    iv = np.exp(-lc)                                   # [D]
    w = mu.astype(np.float64) * iv[None, :]            # [C, D]
    log_det = float(np.sum(lc))
    lp = pl - (np.max(pl) + np.log(np.sum(np.exp(pl - np.max(pl)))))
    mu_sq = np.sum(mu.astype(np.float64) ** 2 * iv[None, :], axis=1)
    cb = (lp - 0.5 * (mu_sq + log_det)).astype(np.float32)      # [C]
    rb = (-0.5 * np.sum(z.astype(np.float64) ** 2 * iv[None, :], axis=1))

    sw = _pow2_scale(float(np.max(np.abs(w))))
    w8 = np.ascontiguousarray((w.T * sw)).astype(ml_dtypes.float8_e4m3)
    sz = _pow2_scale(float(np.max(np.abs(z))))
    scale = np.full((P, 1), 1.0 / (sz * sw), dtype=np.float32)
    cbr = np.ascontiguousarray(cb.reshape(1, C))

    maps = []
    for c in range(NCORES):
        zsh = z[c * NSH:(c + 1) * NSH, :]
        z8c = np.ascontiguousarray(zsh.T * sz).astype(ml_dtypes.float8_e4m3)
        rbc = rb[c * NSH:(c + 1) * NSH].astype(np.float32)
        rbtc = np.ascontiguousarray(rbc.reshape(NT, P).T)       # [P, NT]
        maps.append({"z8": z8c, "w8": w8, "rbt": rbtc, "cbr": cbr,
                     "sc": scale})
    return maps


def _run(z, mu, log_cov_diag, prior_logits, trace=False, **kw):
    nc = _get_nc()
    maps = _in_maps(z, mu, log_cov_diag, prior_logits)
    res = run_bass_kernel_spmd(nc, maps, list(range(NCORES)), trace=trace, **kw)
    full = np.concatenate([res.results[c]["out"] for c in range(NCORES)], axis=0)
    return full, res


def kernel(z, mu, log_cov_diag, prior_logits):
    full, _ = _run(z, mu, log_cov_diag, prior_logits)
    return full
